# revision 1
# baseline (speedup 1.0000x reference)
import os
import sys
import contextlib
import numpy as np

sys.path.insert(0, "/opt/trn_rl_repo")

import concourse.bass as bass  # noqa: E402
import concourse.tile as tile  # noqa: E402
from concourse import bacc, mybir  # noqa: E402
from concourse.bass_utils import run_bass_kernel_spmd  # noqa: E402
from concourse.masks import make_identity  # noqa: E402

FP = mybir.dt.float32
BF = mybir.dt.bfloat16
FR = mybir.dt.float32r
AF = mybir.ActivationFunctionType
ALU = mybir.AluOpType
AX = mybir.AxisListType

# Problem dims (hardcoded; kernel.py must be self-contained)
B, S_FULL, I, H, N, W = 32, 128, 512, 1024, 16384, 128
TH = 3 * H          # 3072
NCORES = 8
BC = B // NCORES    # 4 batches per core
KH = H // 128       # 8
KI = I // 128       # 4
NT = TH // 512      # 6
NC128 = N // 128    # 128 chunks of memory rows
EPS = 1e-8


def _transpose_from_dram(nc, tc, pool_tmp, src_view, dst_tile, n_mchunks,
                         n_kchunks, ident, src_cols):
    """dst[p, k, m] = src[m, k*128+p]; src is [n_mchunks*128, src_cols] in
    DRAM, dst an SBUF tile [128, n_kchunks, n_mchunks*128] (any 4-byte dtype;
    the PSUM->SBUF copy rounds). Uses its own psum pool (2 banks), freed on
    return."""
    with tc.tile_pool(name="tr_ps", bufs=2, space="PSUM") as pps:
        for mj in range(n_mchunks):
            nat = pool_tmp.tile([128, src_cols], FP, tag="tr_nat")
            nc.sync.dma_start(out=nat[:], in_=src_view[mj * 128:(mj + 1) * 128, :])
            for ki in range(n_kchunks):
                tp = pps.tile([128, 128], FP, tag="tr_ps")
                nc.tensor.transpose(tp[:], nat[:, ki * 128:(ki + 1) * 128], ident[:])
                nc.vector.tensor_copy(
                    out=dst_tile[:, ki, mj * 128:(mj + 1) * 128], in_=tp[:])


def _gru_layer(nc, tc, ctx, S, gi_dram, whhT, ident, consts,
               y0T_all=None, bhh_n_tile=None, bhh_pool=None):
    """One GRU layer recurrence over S steps for BC batches.

    gi_dram: [BC, >=S, TH] precomputed input contributions (fp32), with the
        r,z parts of b_hh already folded in when nonzero.
    whhT: SBUF f32r tile [128, KH, TH]
    y0T_all: optional f32r tile [128, KH, BC, S] collecting per-step h.T
    bhh_n_tile: optional [1, TH] fp32 tile, zero outside the n-gate columns
    Returns (h_last [BC, H] fp32 tile, hT_last [128, KH, BC] f32r tile);
    both from pools registered on ctx (alive until ctx closes).
    """
    pgh = ctx.enter_context(tc.tile_pool(name="pgh", bufs=1, space="PSUM"))
    pht = ctx.enter_context(tc.tile_pool(name="pht", bufs=2, space="PSUM"))
    pew = ctx.enter_context(tc.tile_pool(name="pew", bufs=2))
    pgi = ctx.enter_context(tc.tile_pool(name="pgi", bufs=2))
    phh = ctx.enter_context(tc.tile_pool(name="phh", bufs=2))

    h = phh.tile([BC, H], FP, tag="h")
    nc.vector.memset(h[:], 0.0)
    z0 = pew.tile([128, KH, BC], FP, tag="z0")
    nc.vector.memset(z0[:], 0.0)
    hT = phh.tile([128, KH, BC], FR, tag="hT")
    nc.vector.tensor_copy(out=hT[:], in_=z0[:])

    for t in range(S):
        gi = pgi.tile([BC, NT, 512], FP, tag="gi")
        nc.sync.dma_start(out=gi[:], in_=gi_dram[:, t, :].rearrange(
            "b (n x) -> b n x", n=NT))

        pg = pgh.tile([BC, NT, 512], FP, tag="pg")
        for nt in range(NT):
            for k in range(KH):
                nc.tensor.matmul(
                    pg[:, nt, :], hT[:, k, :], whhT[:, k, nt * 512:(nt + 1) * 512],
                    start=(k == 0),
                    stop=(k == KH - 1 and bhh_n_tile is None))
            if bhh_n_tile is not None:
                nc.tensor.matmul(pg[:, nt, :], consts["ones1bc"][:],
                                 bhh_n_tile[:, nt * 512:(nt + 1) * 512],
                                 start=False, stop=True)

        trz = pew.tile([BC, 2048], FP, tag="trz")
        nc.vector.tensor_add(out=trz[:],
                             in0=pg[:, 0:4, :].rearrange("b n x -> b (n x)"),
                             in1=gi[:, 0:4, :].rearrange("b n x -> b (n x)"))
        rz = pew.tile([BC, 2048], FP, tag="trz")
        nc.scalar.activation(out=rz[:], in_=trz[:], func=AF.Sigmoid)
        tn = pew.tile([BC, 1024], FP, tag="t1k")
        nc.vector.tensor_mul(out=tn[:],
                             in0=pg[:, 4:6, :].rearrange("b n x -> b (n x)"),
                             in1=rz[:, 0:1024])
        tn2 = pew.tile([BC, 1024], FP, tag="t1k")
        nc.vector.tensor_add(out=tn2[:], in0=tn[:],
                             in1=gi[:, 4:6, :].rearrange("b n x -> b (n x)"))
        n_g = pew.tile([BC, 1024], FP, tag="t1k")
        nc.scalar.activation(out=n_g[:], in_=tn2[:], func=AF.Tanh)
        hmn = pew.tile([BC, 1024], FP, tag="t1k")
        nc.vector.tensor_tensor(out=hmn[:], in0=h[:], in1=n_g[:], op=ALU.subtract)
        h2 = phh.tile([BC, H], FP, tag="h")
        nc.vector.tensor_mul(out=h2[:], in0=hmn[:], in1=rz[:, 1024:2048])
        nc.vector.tensor_add(out=h2[:], in0=h2[:], in1=n_g[:])

        htp = pht.tile([128, KH, BC], FP, tag="htp")
        for j in range(KH):
            nc.tensor.transpose(htp[:, j, :], h2[:, j * 128:(j + 1) * 128],
                                ident[0:BC, 0:BC])
        hT2 = phh.tile([128, KH, BC], FR, tag="hT")
        nc.vector.tensor_copy(out=hT2[:], in_=htp[:])
        if y0T_all is not None:
            nc.vector.tensor_copy(out=y0T_all[:, :, :, t], in_=hT2[:])
        hT = hT2
        h = h2

    return h, hT


def build_nc(S=S_FULL, nonzero_biases=(), debug=False, stop_phase=7):
    stoph = int(os.environ.get('BASSGRU_STOPH', '9'))
    nzb = set(nonzero_biases)
    nc = bacc.Bacc("TRN2", target_bir_lowering=False, debug=False,
                   num_devices=NCORES)

    # ---- DRAM I/O ----
    x_d = nc.declare_dram_parameter("x", [BC, S_FULL, I], FP, isOutput=False)
    mem_d = nc.declare_dram_parameter("memory", [BC, N, W], FP, isOutput=False)
    wprev_d = nc.declare_dram_parameter("w_prev", [BC, N], FP, isOutput=False)
    Wih0_d = nc.declare_dram_parameter("W_ih0", [TH, I], FP, isOutput=False)
    Whh0_d = nc.declare_dram_parameter("W_hh0", [TH, H], FP, isOutput=False)
    Wih1_d = nc.declare_dram_parameter("W_ih1", [TH, H], FP, isOutput=False)
    Whh1_d = nc.declare_dram_parameter("W_hh1", [TH, H], FP, isOutput=False)
    Wk_d = nc.declare_dram_parameter("Wk", [H, W], FP, isOutput=False)
    Wbeta_d = nc.declare_dram_parameter("Wbeta", [H, 1], FP, isOutput=False)
    Wg_d = nc.declare_dram_parameter("Wg", [H, 1], FP, isOutput=False)
    Wgamma_d = nc.declare_dram_parameter("Wgamma", [H, 1], FP, isOutput=False)
    We_d = nc.declare_dram_parameter("We", [H, W], FP, isOutput=False)
    Wa_d = nc.declare_dram_parameter("Wa", [H, W], FP, isOutput=False)
    Wout_d = nc.declare_dram_parameter("Wout", [I, H + W], FP, isOutput=False)
    bias_d = {}
    for nm, sz in [("bih0", TH), ("bhh0", TH), ("bih1", TH), ("bhh1", TH),
                   ("bk", W), ("bbeta", 1), ("bg", 1), ("bgamma", 1),
                   ("be", W), ("ba", W), ("bout", I)]:
        if nm in nzb:
            bias_d[nm] = nc.declare_dram_parameter(nm, [1, sz], FP,
                                                   isOutput=False)
    out_d = nc.declare_dram_parameter("out", [BC, I], FP, isOutput=True)
    if debug:
        dbg_h1_d = nc.declare_dram_parameter("dbg_h1", [BC, H], FP, isOutput=True)
        dbg_head_d = nc.declare_dram_parameter("dbg_head", [BC, 387], FP,
                                               isOutput=True)
        dbg_w_d = nc.declare_dram_parameter("dbg_w", [128, NC128], FP,
                                            isOutput=True)
        dbg_r_d = nc.declare_dram_parameter("dbg_r", [128, BC], FP, isOutput=True)

    gi0_d = nc.dram_tensor("gi0_scr", [BC, S_FULL, TH], FP)
    gi1_d = nc.dram_tensor("gi1_scr", [BC, S_FULL, TH], FP)
    membf_d = nc.dram_tensor("membf_scr", [BC, N, W], BF)

    with tile.TileContext(nc) as tc, contextlib.ExitStack() as top:
        const = top.enter_context(tc.tile_pool(name="const", bufs=1))
        ptmp = top.enter_context(tc.tile_pool(name="ptmp", bufs=3))

        ident = const.tile([128, 128], FP)
        make_identity(nc, ident[:])
        ones1x128 = const.tile([1, 128], FP)
        nc.vector.memset(ones1x128[:], 1.0)
        ones128 = const.tile([128, 1], FP)
        nc.vector.memset(ones128[:], 1.0)
        onesbf = const.tile([128, 1], BF)
        nc.vector.memset(onesbf[:], 1.0)
        eps128 = const.tile([128, 1], FP)
        nc.vector.memset(eps128[:], EPS)
        ones1bc = const.tile([1, BC], FP)
        nc.vector.memset(ones1bc[:], 1.0)
        consts = {"ones1bc": ones1bc}

        # cast memory fp32 -> bf16 (DRAM->DRAM), early so it overlaps the GRU
        for b in range(BC):
            nc.gpsimd.dma_start(out=membf_d[b], in_=mem_d[b])

        bias_t = {}
        for nm in bias_d:
            t = const.tile([1, bias_d[nm].shape[1]], FP, tag=f"b_{nm}")
            nc.sync.dma_start(out=t[:], in_=bias_d[nm][:])
            bias_t[nm] = t

        def bias_mm(psum_ap, src_ap, nrows):
            nc.tensor.matmul(psum_ap, ones1x128[:, 0:nrows], src_ap,
                             start=False, stop=True)

        # ---------------- phase A0: gi0 = x @ W_ih0.T (+ biases) -------------
        with contextlib.ExitStack() as ph:
            pw = ph.enter_context(tc.tile_pool(name="pw_a0", bufs=1))
            ptb = ph.enter_context(tc.tile_pool(name="ptmp_a0", bufs=2))
            xT = pw.tile([128, KI, BC * S_FULL], FR)
            _transpose_from_dram(nc, tc, ptmp,
                                 x_d[:].rearrange("b s i -> (b s) i"),
                                 xT, BC * S_FULL // 128, KI, ident, I)
            wT = pw.tile([128, KI, TH], FR)
            _transpose_from_dram(nc, tc, ptmp, Wih0_d[:], wT, TH // 128,
                                 KI, ident, I)

            bsum = None
            if "bih0" in nzb or "bhh0" in nzb:
                bsum = pw.tile([1, TH], FP, tag="bsum0")
                nc.vector.memset(bsum[:], 0.0)
                if "bih0" in nzb:
                    nc.vector.tensor_copy(out=bsum[:], in_=bias_t["bih0"][:])
                if "bhh0" in nzb:
                    nc.vector.tensor_add(out=bsum[:, 0:2048],
                                         in0=bsum[:, 0:2048],
                                         in1=bias_t["bhh0"][:, 0:2048])

            with tc.tile_pool(name="pps_a0", bufs=1, space="PSUM") as pps:
                for rj in range(BC * S_FULL // 128):
                    pg = pps.tile([128, NT, 512], FP, tag="pg_a0")
                    for nt in range(NT):
                        for ki in range(KI):
                            nc.tensor.matmul(
                                pg[:, nt, :], xT[:, ki, rj * 128:(rj + 1) * 128],
                                wT[:, ki, nt * 512:(nt + 1) * 512],
                                start=(ki == 0),
                                stop=(ki == KI - 1 and bsum is None))
                        if bsum is not None:
                            bias_mm(pg[:, nt, :],
                                    bsum[:, nt * 512:(nt + 1) * 512], 128)
                    gs = ptb.tile([128, TH], FP, tag="gs_a0")
                    nc.vector.tensor_copy(out=gs[:],
                                          in_=pg[:].rearrange("p n x -> p (n x)"))
                    nc.sync.dma_start(
                        out=gi0_d[:].rearrange("b s h -> (b s) h")[
                            rj * 128:(rj + 1) * 128, :],
                        in_=gs[:])

        # ------- phases A (layer-0) and B (gi1 = y0 @ W_ih1.T) --------------
        if stop_phase >= 2:
          with contextlib.ExitStack() as phy:
              pwy = phy.enter_context(tc.tile_pool(name="pw_y", bufs=1))
              y0T_all = pwy.tile([128, KH, BC, S], FR)

              with contextlib.ExitStack() as ph:
                  pw = ph.enter_context(tc.tile_pool(name="pw_a", bufs=1))
                  whh0T = pw.tile([128, KH, TH], FR)
                  _transpose_from_dram(nc, tc, ptmp, Whh0_d[:], whh0T,
                                       TH // 128, KH, ident, H)
                  bhh0n = None
                  if "bhh0" in nzb:
                      bhh0n = pw.tile([1, TH], FP, tag="bhh0n")
                      nc.vector.memset(bhh0n[:], 0.0)
                      nc.vector.tensor_copy(out=bhh0n[:, 2048:TH],
                                            in_=bias_t["bhh0"][:, 2048:TH])
                  _gru_layer(nc, tc, ph, S, gi0_d, whh0T, ident, consts,
                             y0T_all=y0T_all, bhh_n_tile=bhh0n)

              if stop_phase >= 3:
                with contextlib.ExitStack() as ph:
                    pw = ph.enter_context(tc.tile_pool(name="pw_b", bufs=1))
                    ptb = ph.enter_context(tc.tile_pool(name="ptmp_b", bufs=2))
                    w1T = pw.tile([128, KH, TH], FR)
                    _transpose_from_dram(nc, tc, ptmp, Wih1_d[:], w1T,
                                         TH // 128, KH, ident, H)
                    bsum1 = None
                    if "bih1" in nzb or "bhh1" in nzb:
                        bsum1 = pw.tile([1, TH], FP, tag="bsum1")
                        nc.vector.memset(bsum1[:], 0.0)
                        if "bih1" in nzb:
                            nc.vector.tensor_copy(out=bsum1[:], in_=bias_t["bih1"][:])
                        if "bhh1" in nzb:
                            nc.vector.tensor_add(out=bsum1[:, 0:2048],
                                                 in0=bsum1[:, 0:2048],
                                                 in1=bias_t["bhh1"][:, 0:2048])
                    with tc.tile_pool(name="pps_b", bufs=1, space="PSUM") as pps:
                        for b in range(BC):
                            pg = pps.tile([S, NT, 512], FP, tag="pg_b")
                            for nt in range(NT):
                                for k in range(KH):
                                    nc.tensor.matmul(
                                        pg[:, nt, :], y0T_all[:, k, b, :],
                                        w1T[:, k, nt * 512:(nt + 1) * 512],
                                        start=(k == 0),
                                        stop=(k == KH - 1 and bsum1 is None))
                                if bsum1 is not None:
                                    bias_mm(pg[:, nt, :],
                                            bsum1[:, nt * 512:(nt + 1) * 512], S)
                            gs = ptb.tile([S, TH], FP, tag="gs_b")
                            nc.vector.tensor_copy(
                                out=gs[:], in_=pg[:].rearrange("p n x -> p (n x)"))
                            nc.sync.dma_start(out=gi1_d[b, 0:S, :], in_=gs[:])

        # ---------------- phase C: layer-1 recurrence -----------------------
        if stop_phase >= 4:
          pkeep = top.enter_context(tc.tile_pool(name="pkeep", bufs=1))
          h1T_keep = pkeep.tile([128, KH, BC], FR)
          with contextlib.ExitStack() as ph:
              pw = ph.enter_context(tc.tile_pool(name="pw_c", bufs=1))
              whh1T = pw.tile([128, KH, TH], FR)
              _transpose_from_dram(nc, tc, ptmp, Whh1_d[:], whh1T,
                                   TH // 128, KH, ident, H)
              bhh1n = None
              if "bhh1" in nzb:
                  bhh1n = pw.tile([1, TH], FP, tag="bhh1n")
                  nc.vector.memset(bhh1n[:], 0.0)
                  nc.vector.tensor_copy(out=bhh1n[:, 2048:TH],
                                        in_=bias_t["bhh1"][:, 2048:TH])
              h1, h1T = _gru_layer(nc, tc, ph, S, gi1_d, whh1T, ident, consts,
                                   bhh_n_tile=bhh1n)
              nc.vector.tensor_copy(out=h1T_keep[:], in_=h1T[:])
              if debug:
                  nc.sync.dma_start(out=dbg_h1_d[:], in_=h1[:])

        # ---------------- phase H: NTM head ---------------------------------
        if stop_phase >= 5:
          hp = top.enter_context(tc.tile_pool(name="hp", bufs=1))
          ph_psum_stack = contextlib.ExitStack()
          pps_h = ph_psum_stack.enter_context(
              tc.tile_pool(name="pps_h", bufs=2, space="PSUM"))

          wcatf = hp.tile([128, KH, 512], FP, tag="wcatf")
          nc.vector.memset(wcatf[:], 0.0)
          nc.sync.dma_start(out=wcatf[:, :, 0:128],
                            in_=Wk_d[:].rearrange("(k p) w -> p k w", p=128))
          nc.sync.dma_start(out=wcatf[:, :, 128:256],
                            in_=We_d[:].rearrange("(k p) w -> p k w", p=128))
          nc.sync.dma_start(out=wcatf[:, :, 256:384],
                            in_=Wa_d[:].rearrange("(k p) w -> p k w", p=128))
          nc.sync.dma_start(out=wcatf[:, :, 384:385],
                            in_=Wbeta_d[:].rearrange("(k p) w -> p k w", p=128))
          nc.sync.dma_start(out=wcatf[:, :, 385:386],
                            in_=Wg_d[:].rearrange("(k p) w -> p k w", p=128))
          nc.sync.dma_start(out=wcatf[:, :, 386:387],
                            in_=Wgamma_d[:].rearrange("(k p) w -> p k w", p=128))
          wcat = hp.tile([128, KH, 512], FR, tag="wcat")
          nc.vector.tensor_copy(out=wcat[:], in_=wcatf[:])

          bcat = None
          if any(nm in nzb for nm in ("bk", "bbeta", "bg", "bgamma", "be", "ba")):
              bcat = hp.tile([1, 512], FP, tag="bcat")
              nc.vector.memset(bcat[:], 0.0)
              for nm, lo, hi in [("bk", 0, 128), ("be", 128, 256), ("ba", 256, 384),
                                 ("bbeta", 384, 385), ("bg", 385, 386),
                                 ("bgamma", 386, 387)]:
                  if nm in nzb:
                      nc.vector.tensor_copy(out=bcat[:, lo:hi], in_=bias_t[nm][:])

          phead = pps_h.tile([BC, 512], FP, tag="hps")
          for k in range(KH):
              nc.tensor.matmul(phead[:], h1T_keep[:, k, :], wcat[:, k, :],
                               start=(k == 0),
                               stop=(k == KH - 1 and bcat is None))
          if bcat is not None:
              bias_mm(phead[:], bcat[:], BC)
          head = hp.tile([BC, 512], FP, tag="head")
          nc.vector.tensor_copy(out=head[:], in_=phead[:])
          if debug:
              nc.sync.dma_start(out=dbg_head_d[:], in_=head[:, 0:387])

          if stoph >= 1:
            e_t = hp.tile([BC, 128], FP, tag="e_t")
            nc.scalar.activation(out=e_t[:], in_=head[:, 128:256], func=AF.Sigmoid)
            a_t = hp.tile([BC, 128], FP, tag="a_t")
            nc.scalar.activation(out=a_t[:], in_=head[:, 256:384], func=AF.Tanh)
            # softplus(x) = ln(1 + exp(x)) for beta and gamma (no Softplus table)
            bg2 = hp.tile([BC, 2], FP, tag="bg2")
            nc.scalar.activation(out=bg2[:, 0:1], in_=head[:, 384:385], func=AF.Exp)
            nc.scalar.activation(out=bg2[:, 1:2], in_=head[:, 386:387], func=AF.Exp)
            nc.vector.tensor_scalar_add(bg2[:], bg2[:], 1.0)
            bg2l = hp.tile([BC, 2], FP, tag="bg2l")
            nc.scalar.activation(out=bg2l[:], in_=bg2[:], func=AF.Ln)
            beta_t = hp.tile([BC, 1], FP, tag="beta_t")
            nc.vector.tensor_copy(out=beta_t[:], in_=bg2l[:, 0:1])
            g_t = hp.tile([BC, 1], FP, tag="g_t")
            nc.scalar.activation(out=g_t[:], in_=head[:, 385:386], func=AF.Sigmoid)
            gam_t = hp.tile([BC, 1], FP, tag="gam_t")
            nc.vector.tensor_scalar_add(gam_t[:], bg2l[:, 1:2], 1.0)

          if stoph >= 2:
            k_t = hp.tile([BC, 128], FP, tag="k_t")
            nc.vector.tensor_copy(out=k_t[:], in_=head[:, 0:128])
            kn2 = hp.tile([BC, 1], FP, tag="kn2")
            ksc = hp.tile([BC, 128], FP, tag="ksc")
            nc.vector.tensor_mul(out=ksc[:], in0=k_t[:], in1=k_t[:])
            nc.vector.tensor_reduce(out=kn2[:], in_=ksc[:], axis=AX.X,
                                    op=ALU.add)
            knrm = hp.tile([BC, 1], FP, tag="knrm")
            nc.scalar.activation(out=knrm[:], in_=kn2[:], func=AF.Sqrt)
            nc.vector.tensor_scalar_add(knrm[:], knrm[:], EPS)
            krec = hp.tile([BC, 1], FP, tag="krec")
            nc.vector.reciprocal(out=krec[:], in_=knrm[:])
            nc.vector.tensor_scalar_mul(krec[:], krec[:], beta_t[:])
            kb = hp.tile([BC, 128], FP, tag="kb")
            nc.vector.tensor_scalar_mul(kb[:], k_t[:], krec[:])

          if stoph >= 3:
            def tr_small(src_ap, nrows, ncols, tag):
                tp = pps_h.tile([ncols, nrows], FP, tag="hps_tr")
                nc.tensor.transpose(tp[:], src_ap, ident[0:nrows, 0:nrows])
                dst = hp.tile([ncols, nrows], FP, tag=tag)
                nc.vector.tensor_copy(out=dst[:], in_=tp[:])
                return dst

            kbT = tr_small(kb[:], BC, 128, "kbT")
            eT = tr_small(e_t[:], BC, 128, "eT")
            aT = tr_small(a_t[:], BC, 128, "aT")
            gT = tr_small(g_t[:], BC, 1, "gT")
            gamT = tr_small(gam_t[:], BC, 1, "gamT")

          if stoph >= 4:
            khl = hp.tile([128, 2 * BC], BF, tag="khl")
            nc.vector.tensor_copy(out=khl[:, 0:BC], in_=kbT[:])
            klo = hp.tile([128, BC], FP, tag="klo")
            nc.vector.tensor_tensor(out=klo[:], in0=kbT[:], in1=khl[:, 0:BC],
                                    op=ALU.subtract)
            nc.vector.tensor_copy(out=khl[:, BC:2 * BC], in_=klo[:])

          combT = pkeep.tile([128, KH + 1, BC], FR, tag="combT")
          nc.vector.tensor_copy(out=combT[:, 0:KH, :], in_=h1T_keep[:])

          ph_psum_stack.close()

        # ---------------- SIM + softmax + readpass per batch ----------------
        if stop_phase >= 6:
          with contextlib.ExitStack() as ph:
              psim_pool = ph.enter_context(
                  tc.tile_pool(name="psim", bufs=2, space="PSUM"))
              pcs = ph.enter_context(tc.tile_pool(name="pcs", bufs=2, space="PSUM"))
              prd = ph.enter_context(tc.tile_pool(name="prd", bufs=1, space="PSUM"))
              pmt = ph.enter_context(tc.tile_pool(name="pmt", bufs=3))
              pewq = ph.enter_context(tc.tile_pool(name="pewq", bufs=2))

              def cross_sum(vec128, tag):
                  ps = pcs.tile([1, 1], FP, tag="cs")
                  nc.tensor.matmul(ps[:], vec128, ones128[:], start=True, stop=True)
                  sb = pewq.tile([1, 1], FP, tag=f"css_{tag}")
                  nc.vector.tensor_copy(out=sb[:], in_=ps[:])
                  return sb

              def bcast128(sc11, tag):
                  ps = pcs.tile([128, 1], FP, tag="cs")
                  nc.tensor.matmul(ps[:], ones1x128[:], sc11, start=True, stop=True)
                  sb = pewq.tile([128, 1], FP, tag=f"bcs_{tag}")
                  nc.vector.tensor_copy(out=sb[:], in_=ps[:])
                  return sb

              for b in range(BC):
                  psim = psim_pool.tile([128, NC128, 3], FP, tag="psim")
                  for c2 in range(N // 512):
                      mt = pmt.tile([128, 512], BF, tag="mt")
                      nc.sync.dma_start_transpose(
                          mt[:], membf_d[b, c2 * 512:(c2 + 1) * 512, :])
                      sq = pmt.tile([128, 512], BF, tag="sq")
                      nc.scalar.activation(out=sq[:], in_=mt[:], func=AF.Square)
                      for sub in range(4):
                          cc = c2 * 4 + sub
                          nc.tensor.matmul(psim[:, cc, 0:2],
                                           mt[:, sub * 128:(sub + 1) * 128],
                                           khl[:, b::BC], start=True, stop=True)
                          nc.tensor.matmul(psim[:, cc, 2:3],
                                           sq[:, sub * 128:(sub + 1) * 128],
                                           onesbf[:], start=True, stop=True)

                  psb = pewq.tile([128, NC128, 3], FP, tag="psb")
                  nc.vector.tensor_copy(out=psb[:], in_=psim[:])
                  simd = pewq.tile([128, NC128], FP, tag="simd")
                  nc.vector.tensor_add(out=simd[:], in0=psb[:, :, 0],
                                       in1=psb[:, :, 1])
                  nrm = pewq.tile([128, NC128], FP, tag="nrm")
                  nc.scalar.activation(out=nrm[:], in_=psb[:, :, 2], func=AF.Sqrt)
                  nc.vector.tensor_scalar_add(nrm[:], nrm[:], EPS)
                  rec = pewq.tile([128, NC128], FP, tag="rec")
                  nc.vector.reciprocal(out=rec[:], in_=nrm[:])
                  bs = pewq.tile([128, NC128], FP, tag="bs")
                  nc.vector.tensor_mul(out=bs[:], in0=simd[:], in1=rec[:])
                  es = pewq.tile([128, NC128], FP, tag="es")
                  esum = pewq.tile([128, 1], FP, tag="esum")
                  nc.scalar.activation(out=es[:], in_=bs[:], func=AF.Exp,
                                       accum_out=esum[:])
                  etot = cross_sum(esum[:], "etot")
                  eret = pewq.tile([1, 1], FP, tag="eret")
                  nc.vector.reciprocal(out=eret[:], in_=etot[:])
                  er128 = bcast128(eret[:], "er")
                  wc = pewq.tile([128, NC128], FP, tag="wc")
                  nc.vector.tensor_scalar_mul(wc[:], es[:], er128[:])

                  wpn_nat = pmt.tile([128, 128], FP, tag="wpn_nat")
                  nc.sync.dma_start(out=wpn_nat[:],
                                    in_=wprev_d[b].rearrange("(c p) -> c p", p=128))
                  wpT_ps = prd.tile([128, 128], FP, tag="wpT_ps")
                  nc.tensor.transpose(wpT_ps[:], wpn_nat[:], ident[:])
                  wpT = pewq.tile([128, NC128], FP, tag="wpT")
                  nc.vector.tensor_copy(out=wpT[:], in_=wpT_ps[:])
                  wps = pewq.tile([128, 1], FP, tag="wps")
                  nc.vector.tensor_reduce(out=wps[:], in_=wpT[:], axis=AX.X,
                                          op=ALU.add)
                  wpt = cross_sum(wps[:], "wpt")
                  nc.vector.tensor_scalar_add(wpt[:], wpt[:], EPS)
                  wpr = pewq.tile([1, 1], FP, tag="wpr")
                  nc.vector.reciprocal(out=wpr[:], in_=wpt[:])
                  wpr128 = bcast128(wpr[:], "wpr")
                  wpn = pewq.tile([128, NC128], FP, tag="wpn")
                  nc.vector.tensor_scalar_mul(wpn[:], wpT[:], wpr128[:])

                  gb = bcast128(gT[:, b:b + 1], "gb")
                  dwc = pewq.tile([128, NC128], FP, tag="dwc")
                  nc.vector.tensor_tensor(out=dwc[:], in0=wc[:], in1=wpn[:],
                                          op=ALU.subtract)
                  w0 = pewq.tile([128, NC128], FP, tag="w0")
                  nc.vector.scalar_tensor_tensor(out=w0[:], in0=dwc[:], scalar=gb[:],
                                                 in1=wpn[:], op0=ALU.mult,
                                                 op1=ALU.add)

                  gamb = bcast128(gamT[:, b:b + 1], "gamb")
                  lw = pewq.tile([128, NC128], FP, tag="lw")
                  nc.scalar.activation(out=lw[:], in_=w0[:], func=AF.Ln,
                                       bias=eps128[:])
                  wg = pewq.tile([128, NC128], FP, tag="wg")
                  wgs = pewq.tile([128, 1], FP, tag="wgs")
                  nc.scalar.activation(out=wg[:], in_=lw[:], func=AF.Exp,
                                       scale=gamb[:], accum_out=wgs[:])
                  wgt = cross_sum(wgs[:], "wgt")
                  wgr = pewq.tile([1, 1], FP, tag="wgr")
                  nc.vector.reciprocal(out=wgr[:], in_=wgt[:])
                  wgr128 = bcast128(wgr[:], "wgr")
                  wfin = pewq.tile([128, NC128], FP, tag="wfin")
                  nc.vector.tensor_scalar_mul(wfin[:], wg[:], wgr128[:])
                  if debug and b == 0:
                      nc.sync.dma_start(out=dbg_w_d[:], in_=wfin[:])

                  wsq = pewq.tile([128, NC128], FP, tag="wsq")
                  nc.vector.tensor_mul(out=wsq[:], in0=wfin[:], in1=wfin[:])
                  wss = pewq.tile([128, 1], FP, tag="wss")
                  nc.vector.tensor_reduce(out=wss[:], in_=wsq[:], axis=AX.X,
                                          op=ALU.add)
                  wst = cross_sum(wss[:], "wst")
                  ws128 = bcast128(wst[:], "ws")

                  wv2 = pewq.tile([128, NC128, 2], FP, tag="wv2")
                  nc.vector.tensor_copy(out=wv2[:, :, 0], in_=wfin[:])
                  nc.vector.tensor_copy(out=wv2[:, :, 1], in_=wsq[:])

                  prT = prd.tile([128, 2], FP, tag="prT")
                  for cc in range(NC128):
                      mn = pmt.tile([128, 128], FP, tag="mn")
                      nc.sync.dma_start(out=mn[:],
                                        in_=mem_d[b, cc * 128:(cc + 1) * 128, :])
                      nc.tensor.matmul(prT[:], mn[:], wv2[:, cc, :],
                                       start=(cc == 0), stop=(cc == NC128 - 1))

                  u = pewq.tile([128, 1], FP, tag="u")
                  nc.vector.tensor_mul(out=u[:], in0=prT[:, 1:2], in1=eT[:, b:b + 1])
                  v = pewq.tile([128, 1], FP, tag="v")
                  nc.vector.tensor_tensor(out=v[:], in0=prT[:, 0:1], in1=u[:],
                                          op=ALU.subtract)
                  t5 = pewq.tile([128, 1], FP, tag="t5")
                  nc.vector.tensor_mul(out=t5[:], in0=aT[:, b:b + 1], in1=ws128[:])
                  rcol = pewq.tile([128, 1], FP, tag="rcol")
                  nc.vector.tensor_add(out=rcol[:], in0=v[:], in1=t5[:])
                  nc.vector.tensor_copy(out=combT[:, KH, b:b + 1], in_=rcol[:])

        # ---------------- phase OUT -----------------------------------------
        if stop_phase >= 7:
          with contextlib.ExitStack() as ph:
              pw = ph.enter_context(tc.tile_pool(name="pw_o", bufs=1))
              pps_o = ph.enter_context(tc.tile_pool(name="pps_o", bufs=1,
                                                    space="PSUM"))
              woutT = pw.tile([128, KH + 1, I], FR)
              _transpose_from_dram(nc, tc, ptmp, Wout_d[:], woutT, I // 128,
                                   KH + 1, ident, H + W)
              po = pps_o.tile([BC, I], FP, tag="po")
              for k in range(KH + 1):
                  nc.tensor.matmul(po[:], combT[:, k, :], woutT[:, k, :],
                                   start=(k == 0),
                                   stop=(k == KH and "bout" not in nzb))
              if "bout" in nzb:
                  bias_mm(po[:], bias_t["bout"][:], BC)
              ob = pw.tile([BC, I], FP, tag="ob")
              nc.vector.tensor_copy(out=ob[:], in_=po[:])
              nc.sync.dma_start(out=out_d[:], in_=ob[:])
              if debug:
                  rTdbg = pw.tile([128, BC], FP, tag="rTdbg")
                  nc.vector.tensor_copy(out=rTdbg[:], in_=combT[:, KH, :])
                  nc.sync.dma_start(out=dbg_r_d[:], in_=rTdbg[:])

        else:
            zo = ptmp.tile([BC, I], FP, tag='zo')
            nc.vector.memset(zo[:], 0.0)
            nc.sync.dma_start(out=out_d[:], in_=zo[:])
    nc.compile()
    return nc


_NC_CACHE = {}


def _get_nc(S, nzb_key, debug):
    sp = int(os.environ.get('BASSGRU_STOP', '7'))
    key = (S, nzb_key, debug, sp)
    if key not in _NC_CACHE:
        _NC_CACHE[key] = build_nc(S=S, nonzero_biases=nzb_key, debug=debug, stop_phase=sp)
    return _NC_CACHE[key]


def make_in_maps(inputs, S=S_FULL, debug=False):
    f32 = lambda a: np.ascontiguousarray(np.asarray(a), dtype=np.float32)
    bias_names = {"bih0": "b_ih0", "bhh0": "b_hh0", "bih1": "b_ih1",
                  "bhh1": "b_hh1", "bk": "bk", "bbeta": "bbeta", "bg": "bg",
                  "bgamma": "bgamma", "be": "be", "ba": "ba", "bout": "bout"}
    nzb = tuple(sorted(k for k, src in bias_names.items()
                       if np.any(np.asarray(inputs[src]) != 0)))
    nc = _get_nc(S, nzb, debug)
    shared = {nm: f32(inputs[nm]) for nm in
              ["W_ih0", "W_hh0", "W_ih1", "W_hh1", "Wk", "Wbeta", "Wg",
               "Wgamma", "We", "Wa", "Wout"]}
    for k, src in bias_names.items():
        if k in nzb:
            shared[k] = f32(inputs[src]).reshape(1, -1)
    x = f32(inputs["x"])
    mem = f32(inputs["memory"])
    wp = f32(inputs["w_prev"])
    in_maps = []
    for c in range(NCORES):
        m = dict(shared)
        m["x"] = x[c * BC:(c + 1) * BC]
        m["memory"] = mem[c * BC:(c + 1) * BC]
        m["w_prev"] = wp[c * BC:(c + 1) * BC]
        in_maps.append(m)
    return nc, in_maps, nzb


def kernel(**inputs) -> np.ndarray:
    debug = bool(int(os.environ.get("BASSGRU_DEBUG", "0")))
    S = int(os.environ.get("BASSGRU_S", str(S_FULL)))
    nc, in_maps, _ = make_in_maps(inputs, S=S, debug=debug)
    res = run_bass_kernel_spmd(nc, in_maps, list(range(NCORES)))
    outs = [res.results[c]["out"] for c in range(NCORES)]
    if debug:
        kernel.last_results = res.results
    return np.concatenate(outs, axis=0).astype(np.float32)

